# revision 25
# baseline (speedup 1.0000x reference)
"""DyConvAtten Trainium2 kernel.

Computation (per batch sample b):
  weight[n, c] = f[b] @ W_lin.T + b_lin          # [N, N*K] dynamic conv weights
  w4[o, i, t]  = weight[o, i*K + t]
  y[o, h]      = sum_{i,t} w4[o, i, t] * x[b, i, h + t - 1]   (zero-padded)
  out[b]       = LayerNorm_H(y) * gamma + beta

Strategy: data-parallel over batch B=64 across 8 NeuronCores (8 samples/core).
Per sample, both stages are matmuls on TensorE:
  mm1: lhsT = W_lin^T tap-slices [H, N] chunks, rhs = f[b]^T [H, N] chunks
       -> psum_w[t][i, o] (the conv weights, already transposed for mm2)
  mm2: lhsT = psum_w tap slices [i, o] chunks, rhs = shifted x[b] [i, H]
       -> psum_y[o, h], then LayerNorm via bn_stats/bn_aggr + ScalarE apply.

Host-side prep: transpose f to [H, N] layout, regroup W_lin by tap, cast to
the compute dtype. This is one-time numpy work outside the measured kernel.
"""

import os

import numpy as np
import ml_dtypes

import concourse.bass as bass
import concourse.bacc as bacc_mod
import concourse.tile as tile
import concourse.mybir as mybir
from concourse.bass_utils import run_bass_kernel_spmd

B, N, H, K = 64, 256, 1024, 3
NCORES = 8
S = B // NCORES  # samples per core
LN_EPS = 1e-5
HC = H // 128  # h chunks (8)
IC = N // 128  # in-channel chunks (2)
OC = N // 128  # out-channel chunks (2)
HH = H // 512  # free-dim halves for psum_y (2)

# compute dtype: "bf16" (fast, ~4e-3 scale-relative err) | "fp32r"
# (reduced-precision fp32 matmul mode, ~2e-4, ~11% slower) | "fp32"
# (exact, 4x slower matmul)
COMPUTE_DT = os.environ.get("DYCONV_DT", "bf16")

_cache = {}


def _build_program(dt_str: str, bias_zero: bool, gb_trivial: bool):
    f32 = mybir.dt.float32
    if dt_str == "bf16":
        dtc = mybir.dt.bfloat16
    elif dt_str == "fp32r":
        # reduced-precision full-rate fp32 matmul mode; operand tiles (and
        # their DRAM sources) must be declared float32r for the BIR verifier
        dtc = mybir.dt.float32r
    elif dt_str == "fp32":
        dtc = f32
    else:
        raise ValueError(dt_str)
    mm = lambda ap: ap  # noqa: E731

    AF = mybir.ActivationFunctionType
    OP = mybir.AluOpType

    nc = bacc_mod.Bacc("TRN2")
    # [s, p, hc, o] : f[b].T chunked so contraction dim h sits on partitions
    fT = nc.dram_tensor("fT", [S, 128, HC, N], dtc, kind="ExternalInput")
    # [s, ic, p, h] : k[b] with in-channel i on partitions
    kx = nc.dram_tensor("kx", [S, IC, 128, H], dtc, kind="ExternalInput")
    # [p, t, hc, i] : W_lin regrouped by tap, h on partitions
    wt = nc.dram_tensor("wt", [128, K, HC, N], dtc, kind="ExternalInput")
    out = nc.dram_tensor("out", [S, OC, 128, H], f32, kind="ExternalOutput")
    if not bias_zero:
        bl = nc.dram_tensor("bl", [K, IC, 128], f32, kind="ExternalInput")
    if not gb_trivial:
        gm = nc.dram_tensor("gm", [H], f32, kind="ExternalInput")
        bt = nc.dram_tensor("bt", [H], f32, kind="ExternalInput")

    with tile.TileContext(nc) as tc:
        with (
            tc.tile_pool(name="const", bufs=1) as const,
            tc.tile_pool(name="io", bufs=3) as io,
            tc.tile_pool(name="mid", bufs=2) as mid,
            tc.tile_pool(name="small", bufs=4) as small,
            tc.tile_pool(name="psw", bufs=1, space="PSUM") as psw,
            tc.tile_pool(name="psy", bufs=1, space="PSUM") as psy,
        ):
            # PE warmup: a few matmuls on zeroed bf16 tiles, queued before any
            # DMA-gated work, so the HAM clock-gate releases (1.2 -> 2.4 GHz)
            # while the first input transfers are still in flight.
            bf16 = mybir.dt.bfloat16
            warm_a = const.tile([128, 128], bf16)
            warm_b = const.tile([128, 512], bf16)
            nc.gpsimd.memset(warm_a[:], 0.0)
            nc.gpsimd.memset(warm_b[:], 0.0)
            n_warm = 10 if dt_str == "bf16" else 8
            with tc.tile_pool(name="pswarm", bufs=1, space="PSUM") as pswarm:
                warm_p = pswarm.tile([128, 512], f32)
                for _ in range(n_warm):
                    nc.tensor.matmul(
                        warm_p[:], warm_a[:], warm_b[:], start=True, stop=True
                    )

            wt_sb = const.tile([128, K, HC, N], dtc)
            eps_t = const.tile([128, 1], f32)
            nc.vector.memset(eps_t[:], LN_EPS)
            if not bias_zero:
                bl_sb = const.tile([128, K, IC], f32)
                nc.sync.dma_start(
                    out=bl_sb[:], in_=bl[:, :, :].rearrange("t ic p -> p t ic")
                )
            if not gb_trivial:
                gm_sb = const.tile([128, H], f32)
                nc.sync.dma_start(
                    out=gm_sb[:],
                    in_=bass.AP(tensor=gm[:].tensor, offset=0, ap=[[0, 128], [1, H]]),
                )
                bt_sb = const.tile([128, H], f32)
                nc.sync.dma_start(
                    out=bt_sb[:],
                    in_=bass.AP(tensor=bt[:].tensor, offset=0, ap=[[0, 128], [1, H]]),
                )

            for s in range(S):
                ft_t = io.tile([128, HC, N], dtc, tag="ft")
                if s == 0:
                    # first-sample loads: ft on the Sync DGE, per-tap weight
                    # chunks on the Scalar DGE so dispatch costs (~0.65us
                    # each) run in parallel; mm1's tap-0 group gates on just
                    # ft + tap 0 while taps 1/2 land during tap 0's matmuls.
                    nc.sync.dma_start(out=ft_t[:, 0:4], in_=fT[s, :, 0:4])
                    nc.scalar.dma_start(out=wt_sb[:, 0, 0:4], in_=wt[:, 0, 0:4])
                    nc.sync.dma_start(out=ft_t[:, 4:8], in_=fT[s, :, 4:8])
                    nc.scalar.dma_start(out=wt_sb[:, 0, 4:8], in_=wt[:, 0, 4:8])
                    nc.scalar.dma_start(out=wt_sb[:, 1], in_=wt[:, 1])
                    nc.scalar.dma_start(out=wt_sb[:, 2], in_=wt[:, 2])
                else:
                    nc.sync.dma_start(out=ft_t[:], in_=fT[s])
                kx_t = io.tile([128, IC, H + 2], dtc, tag="kx")
                ms = lambda ap: (  # noqa: E731
                    ap.bitcast(f32) if dtc == mybir.dt.float32r else ap
                )
                nc.gpsimd.memset(ms(kx_t[:, :, 0:1]), 0.0)
                nc.gpsimd.memset(ms(kx_t[:, :, H + 1 : H + 2]), 0.0)
                nc.sync.dma_start(
                    out=kx_t[:, :, 1 : H + 1],
                    in_=kx[s].rearrange("ic p h -> p ic h"),
                )

                # ---- mm1: dynamic weight generation -> psum_w[t][i, o] ----
                psw_t = [
                    psw.tile([128, IC, N], f32, tag=f"w{t}", name=f"psw{t}") for t in range(K)
                ]
                w_sb = mid.tile([128, K, IC, N], dtc, tag="wsb")
                for t in range(K):
                    for ic in range(IC):
                        for hc in range(HC):
                            nc.tensor.matmul(
                                psw_t[t][:, ic, :],
                                mm(wt_sb[:, t, hc, ic * 128 : (ic + 1) * 128]),
                                mm(ft_t[:, hc, :]),
                                start=(hc == 0),
                                stop=(hc == HC - 1),
                            )
                    # PSUM -> SBUF (dtype cast + optional b_lin bias)
                    if bias_zero:
                        nc.scalar.activation(
                            out=w_sb[:, t], in_=psw_t[t][:, :, :], func=AF.Copy
                        )
                    else:
                        for ic in range(IC):
                            nc.scalar.activation(
                                out=w_sb[:, t, ic],
                                in_=psw_t[t][:, ic, :],
                                func=AF.Identity,
                                bias=bl_sb[:, t, ic : ic + 1],
                            )

                # ---- mm2: dynamic conv -> psum_y[oc][hh] ----
                psy_t = [
                    psy.tile([128, HH, 512], f32, tag=f"y{oc}", name=f"psy{oc}") for oc in range(OC)
                ]
                out_sb = mid.tile([128, OC, H], f32, tag="osb")
                for oc in range(OC):
                    for hh in range(HH):
                        n_mm = 0
                        for t in range(K):
                            for ic in range(IC):
                                n_mm += 1
                                nc.tensor.matmul(
                                    psy_t[oc][:, hh, :],
                                    mm(w_sb[:, t, ic, oc * 128 : (oc + 1) * 128]),
                                    mm(kx_t[:, ic, hh * 512 + t : hh * 512 + t + 512]),
                                    start=(n_mm == 1),
                                    stop=(n_mm == K * IC),
                                )

                # ---- LayerNorm over h ----
                for oc in range(OC):
                    stats = small.tile([128, HH, 6], f32, tag="stats")
                    for hh in range(HH):
                        nc.vector.bn_stats(
                            out=stats[:, hh, :], in_=psy_t[oc][:, hh, :]
                        )
                    mv = small.tile([128, 2], f32, tag="mv")
                    nc.vector.bn_aggr(out=mv[:], in_=stats[:])
                    s_t = small.tile([128, 1], f32, tag="s")
                    nc.scalar.activation(
                        out=s_t[:], in_=mv[:, 1:2], func=AF.Sqrt, bias=eps_t[:]
                    )
                    nc.vector.reciprocal(out=s_t[:], in_=s_t[:])
                    nb = small.tile([128, 1], f32, tag="nb")
                    nc.vector.tensor_scalar(
                        out=nb[:],
                        in0=mv[:, 0:1],
                        scalar1=s_t[:],
                        scalar2=-1.0,
                        op0=OP.mult,
                        op1=OP.mult,
                    )
                    # apply (y - mu) * rsqrt split across engines: hh0 on
                    # VectorE, hh1 on ScalarE, so the two halves run in
                    # parallel instead of serializing on ACT
                    nc.vector.tensor_scalar(
                        out=out_sb[:, oc, 0:512],
                        in0=psy_t[oc][:, 0, :],
                        scalar1=mv[:, 0:1],
                        scalar2=s_t[:],
                        op0=OP.subtract,
                        op1=OP.mult,
                    )
                    nc.scalar.activation(
                        out=out_sb[:, oc, 512:1024],
                        in_=psy_t[oc][:, 1, :],
                        func=AF.Identity,
                        bias=nb[:],
                        scale=s_t[:],
                    )
                    if not gb_trivial:
                        nc.vector.tensor_mul(
                            out_sb[:, oc], out_sb[:, oc], gm_sb[:]
                        )
                        nc.vector.tensor_add(
                            out_sb[:, oc], out_sb[:, oc], bt_sb[:]
                        )
                        nc.sync.dma_start(out=out[s, oc], in_=out_sb[:, oc])
                    else:
                        # per-half writeback: each half's transfer starts as
                        # soon as its apply finishes
                        nc.sync.dma_start(
                            out=out[s, oc, :, 0:512], in_=out_sb[:, oc, 0:512]
                        )
                        nc.sync.dma_start(
                            out=out[s, oc, :, 512:1024],
                            in_=out_sb[:, oc, 512:1024],
                        )

    nc.compile()
    return nc


def _np_dt(dt_str):
    return ml_dtypes.bfloat16 if dt_str == "bf16" else np.float32


def run(inputs, trace=False, dt_str=None, trace_kwargs={}):
    dt_str = dt_str or COMPUTE_DT
    f = np.asarray(inputs["f"], dtype=np.float32)
    k = np.asarray(inputs["k"], dtype=np.float32)
    W_lin = np.asarray(inputs["W_lin"], dtype=np.float32)
    b_lin = np.asarray(inputs["b_lin"], dtype=np.float32)
    gamma = np.asarray(inputs["ln_gamma"], dtype=np.float32)
    beta = np.asarray(inputs["ln_beta"], dtype=np.float32)

    bias_zero = not b_lin.any()
    gb_trivial = bool(np.all(gamma == 1.0) and not beta.any())

    key = (dt_str, bias_zero, gb_trivial)
    if key not in _cache:
        _cache[key] = _build_program(*key)
    nc = _cache[key]

    npdt = _np_dt(dt_str)
    # fT[s, p, hc, o] = f[s, o, hc*128 + p]
    fT = np.ascontiguousarray(
        f.reshape(B, N, HC, 128).transpose(0, 3, 2, 1), dtype=npdt
    )
    # kx[s, ic, p, h] = k[s, ic*128 + p, h]
    kx = np.ascontiguousarray(k.reshape(B, IC, 128, H), dtype=npdt)
    # wt[p, t, hc, i] = W_lin[i*K + t, hc*128 + p]
    wt = np.ascontiguousarray(
        W_lin.reshape(N, K, HC, 128).transpose(3, 1, 2, 0), dtype=npdt
    )

    in_maps = []
    for c in range(NCORES):
        m = {
            "fT": fT[c * S : (c + 1) * S],
            "kx": kx[c * S : (c + 1) * S],
            "wt": wt,
        }
        if not bias_zero:
            # bl[t, ic, p] = b_lin[(ic*128 + p)*K + t]
            m["bl"] = np.ascontiguousarray(
                b_lin.reshape(IC, 128, K).transpose(2, 0, 1), dtype=np.float32
            )
        if not gb_trivial:
            m["gm"] = gamma
            m["bt"] = beta
        in_maps.append(m)

    res = run_bass_kernel_spmd(
        nc,
        in_maps,
        core_ids=list(range(NCORES)),
        trace=trace,
        trace_kwargs=trace_kwargs,
    )
    outs = [res.results[c]["out"].reshape(S, N, H) for c in range(NCORES)]
    return np.concatenate(outs, axis=0), res


def kernel(**inputs) -> np.ndarray:
    out, _ = run(inputs, trace=False)
    return out
